# revision 1
# baseline (speedup 1.0000x reference)
"""Elman RNN on 8 trn2 cores, data-parallel over batch.

h_t = tanh(x_t @ w_i + h_{t-1} @ w_h + b_h), shapes L=512, N=128, D=256, H=512.

Per core (N_c = 16 samples): keep h transposed (h^T: H on partitions, batch on
free). Phase 1 precomputes xi^T = w_i^T x^T + b_h for all steps (fp16, resident
in SBUF). Phase 2 runs the 512-step recurrence: per step, ACT prefills a PSUM
tile z with xi (Identity, one step ahead so it hides under the matmuls), 16
matmuls (w_h 128x128 fp16 blocks stationary, h^T chunks moving) accumulate into
z, then one ACT Tanh writes h^T back to SBUF fp16. Output is re-transposed to
natural layout via the PE in 32-step stages into a write-once staging buffer
and DMA'd out with an fp16->fp32 cast.

Walrus permits at most ONE sem wait per instruction and wait elision is purely
per-engine history (no transitive reasoning), so: PSUM tiles are framework ring
tiles (one bank each; a write prunes the bank's dep history), transposes split
their two deps across ldweights/matmult naturally, and one-time ldweights /
tiny-copy observers pre-load each engine's wait history where a second
cross-engine dep would otherwise appear.
"""

import numpy as np

import concourse.bass as bass
import concourse.mybir as mybir
import concourse.tile as tile
from concourse.bass_utils import run_bass_kernel_spmd
from concourse.masks import make_identity

L, N, D, H = 512, 128, 256, 512
NCORES = 8
NC = N // NCORES        # samples per core
R = L * NC              # (t, n) rows per core
FCH = 512               # (t, n) elements per xi chunk / output stage
NF = R // FCH
TST = FCH // NC         # steps per output stage
NST = L // TST
FP32 = mybir.dt.float32
FP16 = mybir.dt.float16
AF = mybir.ActivationFunctionType

_cache = {}


def _build():
    nc = bass.Bass("TRN2", debug=False)
    x_d = nc.dram_tensor("x", [R, D], FP32, kind="ExternalInput").ap()
    wi_d = nc.dram_tensor("w_i", [D, H], FP32, kind="ExternalInput").ap()
    wh_d = nc.dram_tensor("w_h", [H, H], FP32, kind="ExternalInput").ap()
    bh_d = nc.dram_tensor("b_h", [H], FP32, kind="ExternalInput").ap()
    out_d = nc.dram_tensor("h_out", [R, H], FP32, kind="ExternalOutput").ap()

    with tile.TileContext(nc) as tc:
        with (
            tc.tile_pool(name="const", bufs=1) as cp,
            tc.tile_pool(name="work", bufs=2) as wp,
            tc.tile_pool(name="ps", bufs=2, space="PSUM") as pp,
        ):
            ident = cp.tile([128, 128], FP16, tag="ident")
            make_identity(nc, ident)

            # Weights + x cast fp32->fp16 in-flight by SWDGE into write-once
            # buffers.
            wh = []
            for k in range(4):
                whk = cp.tile([128, H], FP16, tag=f"wh{k}", name=f"wh{k}")
                nc.gpsimd.dma_start(whk, wh_d[k * 128 : (k + 1) * 128, :])
                wh.append(whk)
            wi = []
            for k in range(2):
                wik = cp.tile([128, H], FP16, tag=f"wi{k}", name=f"wi{k}")
                nc.gpsimd.dma_start(wik, wi_d[k * 128 : (k + 1) * 128, :])
                wi.append(wik)
            bh = cp.tile([128, 4], FP32, tag="bh")
            nc.gpsimd.dma_start(bh, bh_d.rearrange("(m p) -> p m", p=128))

            # all of x, fp16, row-tile-major: 64 tiles of [128, D]
            xall = cp.tile([128, (R // 128) * D], FP16, tag="xall")
            xall_r = xall.rearrange("p (rt d) -> p rt d", d=D)
            x_r = x_d.rearrange("(rt p) d -> p rt d", p=128)
            for f in range(NF):
                nc.gpsimd.dma_start(
                    xall_r[:, f * 4 : (f + 1) * 4, :], x_r[:, f * 4 : (f + 1) * 4, :]
                )

            # xi^T, m-major on the free axis: [:, m*R + t*NC + n]
            xi = cp.tile([128, 4 * R], FP16, tag="xi")
            xi_r = xi.rearrange("p (m r) -> p m r", m=4)
            dscr = cp.tile([128, NF], FP16, tag="dscr")
            ascr = cp.tile([128, NST], FP16, tag="ascr")

            # write-once natural-layout output staging (one region per stage)
            nat = cp.tile([128, NST * 4 * H], FP16, tag="nat")
            pscr = cp.tile([128, NST], FP16, tag="pscr")

            # One-time observers: PE observes Pool (ident) and the wi DMA
            # lanes; DVE observes the bh DMA lane.
            nc.tensor.ldweights(ident)
            nc.tensor.ldweights(wi[0][:, :128])
            nc.tensor.ldweights(wi[1][:, :128])
            bhobs = cp.tile([128, 4], FP32, tag="bhobs")
            nc.vector.tensor_copy(bhobs, bh)

            # ---- phase 1: x^T tiles + xi = w_i^T x^T + b_h ----
            xts_prev = None
            for f in range(NF):
                if xts_prev is not None:
                    # DVE observes its own sem at the last copy of f-1 so the
                    # ring-WAW self-waits of this f's copies elide.
                    nc.vector.tensor_copy(
                        dscr[:, f : f + 1], xts_prev[1][:, FCH - 1 :]
                    )
                xts = []
                for kd in range(2):
                    xt = wp.tile([128, FCH], FP16, tag=f"xT{kd}", name=f"xT{kd}_{f}")
                    xts.append(xt)
                for rt in range(4):
                    base = (f * 4 + rt) * D
                    for kd in range(2):
                        tp = pp.tile(
                            [128, 128], FP16, tag="tp", bufs=4, name=f"tp{f}_{rt}_{kd}"
                        )
                        nc.tensor.transpose(
                            tp, xall[:, base + kd * 128 : base + (kd + 1) * 128], ident
                        )
                        nc.vector.tensor_copy(xts[kd][:, rt * 128 : (rt + 1) * 128], tp)
                for m in range(4):
                    xps = pp.tile([128, FCH], FP32, tag="xips", name=f"xps{f}_{m}")
                    for kd in range(2):
                        nc.tensor.matmul(
                            xps,
                            wi[kd][:, m * 128 : (m + 1) * 128],
                            xts[kd],
                            start=(kd == 0),
                            stop=(kd == 1),
                        )
                    nc.vector.tensor_scalar_add(
                        xi[:, m * R + f * FCH : m * R + (f + 1) * FCH],
                        xps,
                        bh[:, m : m + 1],
                    )
                xts_prev = xts

            # ---- phase 2: recurrence + output staging ----
            z_cur = None
            h_prev = None
            h_acc_prev = None
            for s in range(NST):
                if s >= 2:
                    # ACT observes its own sem at the last tanh of s-1 so the
                    # hacc ring WAW of this stage's first tanh elides.
                    nc.scalar.activation(
                        ascr[:, s : s + 1], h_acc_prev[:, 4 * FCH - 1 :], AF.Identity
                    )
                h_acc = wp.tile([128, 4 * FCH], FP16, tag="hacc", name=f"hacc{s}")
                h_acc_r = h_acc.rearrange("p (m fc) -> p m fc", m=4)
                for tl in range(TST):
                    t = s * TST + tl
                    if t > 0:
                        for m in range(4):
                            for k in range(4):
                                nc.tensor.matmul(
                                    z_cur[:, m * 16 : (m + 1) * 16],
                                    wh[k][:, m * 128 : (m + 1) * 128],
                                    h_prev(k),
                                    start=False,
                                    stop=(k == 3),
                                    skip_group_check=True,
                                )
                    # prefill z for step t+1 via PE (ident stationary, xi
                    # moving, start=True): the whole prefill+accumulate chain
                    # stays on one engine, and emitting it after step t's MMs
                    # lets their ACT wait cover the z-slot read WAR.
                    if t + 1 < L:
                        z_next = pp.tile([128, 64], FP32, tag="z", name=f"z{t + 1}")
                        nc.tensor.matmul(
                            z_next,
                            ident,
                            xi_r[:, :, (t + 1) * NC : (t + 2) * NC],
                            start=True,
                            stop=False,
                            skip_group_check=True,
                        )
                    else:
                        z_next = None
                    out_sl = h_acc_r[:, :, tl * NC : (tl + 1) * NC]
                    if t == 0:
                        nc.scalar.activation(out_sl, xi_r[:, :, :NC], AF.Tanh)
                    else:
                        nc.scalar.activation(
                            out_sl, z_cur.rearrange("p (m w) -> p m w", m=4), AF.Tanh
                        )
                    h_prev = (
                        lambda ha, tl_: lambda k: ha[
                            :, k * FCH + tl_ * NC : k * FCH + (tl_ + 1) * NC
                        ]
                    )(h_acc, tl)
                    z_cur = z_next

                # ---- output staging for stage s ----
                nb = s * 4 * H
                for rt in range(4):
                    for m in range(4):
                        otp = pp.tile(
                            [128, 128], FP16, tag="tp", bufs=4, name=f"otp{s}_{rt}_{m}"
                        )
                        nc.tensor.transpose(
                            otp,
                            h_acc[:, m * FCH + rt * 128 : m * FCH + (rt + 1) * 128],
                            ident,
                        )
                        nc.vector.tensor_copy(
                            nat[:, nb + rt * H + m * 128 : nb + rt * H + (m + 1) * 128],
                            otp,
                        )
                # Pool observes DVE at the last nat copy so the out DMA only
                # needs its DMASW chain wait.
                nc.gpsimd.tensor_copy(pscr[:, s : s + 1], nat[:, nb + 4 * H - 1 :][:, :1])
                nc.gpsimd.dma_start(
                    out_d[s * FCH : (s + 1) * FCH, :].rearrange(
                        "(rt p) h -> p rt h", p=128
                    ),
                    nat[:, nb : nb + 4 * H].rearrange("p (rt h) -> p rt h", h=H),
                )
                h_acc_prev = h_acc
    _split_waits(nc)
    return nc


def _split_waits(nc):
    # Walrus accepts at most one sem wait per instruction, but the TileContext
    # end-of-program drain aggregates every sem's terminal value. Split any
    # multi-wait instruction into a chain of single-wait drains ahead of it
    # (same engine, in-order issue => identical semantics).
    for f in nc.m.functions:
        for blk in f.blocks:
            insts = list(blk.instructions)
            out = []
            changed = False
            for ins in insts:
                si = ins.sync_info
                w = list(si.on_wait) if si is not None else []
                if len(w) > 1:
                    changed = True
                    for k, sw in enumerate(w[:-1]):
                        nd = mybir.InstDrain(name=f"{ins.name}-w{k}", ins=[], outs=[])
                        nd.engine = ins.engine
                        nd.sync_info = mybir.SyncInfo(on_wait=[sw], on_update=[])
                        out.append(nd)
                    ins.sync_info = mybir.SyncInfo(
                        on_wait=[w[-1]], on_update=list(ins.sync_info.on_update)
                    )
                out.append(ins)
            if changed:
                blk.instructions = out


def _get_nc():
    if "nc" not in _cache:
        _cache["nc"] = _build()
    return _cache["nc"]


def run(inputs, **spmd_kwargs):
    x = np.ascontiguousarray(np.asarray(inputs["x"], dtype=np.float32))
    w_i = np.ascontiguousarray(np.asarray(inputs["w_i"], dtype=np.float32))
    w_h = np.ascontiguousarray(np.asarray(inputs["w_h"], dtype=np.float32))
    b_h = np.ascontiguousarray(np.asarray(inputs["b_h"], dtype=np.float32))
    in_maps = []
    for c in range(NCORES):
        xs = np.ascontiguousarray(x[:, c * NC : (c + 1) * NC, :]).reshape(R, D)
        in_maps.append({"x": xs, "w_i": w_i, "w_h": w_h, "b_h": b_h})
    res = run_bass_kernel_spmd(_get_nc(), in_maps, list(range(NCORES)), **spmd_kwargs)
    out = np.empty((L, N, H), np.float32)
    for c in range(NCORES):
        out[:, c * NC : (c + 1) * NC, :] = res.results[c]["h_out"].reshape(L, NC, H)
    return out, res


def kernel(**inputs) -> np.ndarray:
    out, _ = run(inputs)
    return out



# revision 2
# speedup vs baseline: 1.0023x; 1.0023x over previous
"""Elman RNN on 8 trn2 cores, data-parallel over batch — fused-pipeline version.

h_t = tanh(x_t @ w_i + h_{t-1} @ w_h + b_h), shapes L=512, N=128, D=256, H=512.

Per core (N_c = 16 samples), h is kept transposed (h^T: H interleaved on
partitions as [p, (m, n)], m = H//128 block). The 512-step recurrence is the
critical path (~0.8us/step: 16 accumulating matmuls + 1 PSUM prefill + 1
tanh); everything else hides inside its idle windows:

- xi^T = w_i^T x^T + b_h is computed stage-by-stage *inside* the recurrence
  stream: while the recurrence runs stage s (32 steps), the PE transposes and
  matmuls for xi chunk s+2 are dribbled one-per-step into the tanh-wait
  windows. Only chunks 0,1 are computed up front (~10us prologue).
- The output is written in h^T layout ([H, L*NC] per core, fp16->fp32 cast in
  the DMA) one stage at a time; the host transposes back during unshard. No
  on-device re-transpose, no staging copies.

All ACT instructions are Tanh (incl. the ring-WAW observer) so the activation
table is loaded exactly once.

Walrus permits at most ONE sem wait per instruction; wait elision is purely
per-engine history. Ring tiles + tiny observer ops keep each instruction to a
single wait; _split_waits() splits any stragglers (notably the end-of-program
drain) into single-wait drain chains.
"""

import numpy as np

import concourse.bass as bass
import concourse.mybir as mybir
import concourse.tile as tile
from concourse.bass_utils import run_bass_kernel_spmd
from concourse.masks import make_identity

L, N, D, H = 512, 128, 256, 512
NCORES = 8
NC = N // NCORES        # samples per core
R = L * NC              # (t, n) rows per core
FCH = 512               # (t, n) elements per xi chunk / output stage
NF = R // FCH           # 16 chunks
TST = FCH // NC         # 32 steps per stage
NST = L // TST          # 16 stages
FP32 = mybir.dt.float32
FP16 = mybir.dt.float16
AF = mybir.ActivationFunctionType

_cache = {}


def _build():
    nc = bass.Bass("TRN2", debug=False)
    x_d = nc.dram_tensor("x", [R, D], FP32, kind="ExternalInput").ap()
    wi_d = nc.dram_tensor("w_i", [D, H], FP32, kind="ExternalInput").ap()
    wh_d = nc.dram_tensor("w_h", [H, H], FP32, kind="ExternalInput").ap()
    bh_d = nc.dram_tensor("b_h", [H], FP32, kind="ExternalInput").ap()
    # transposed output: row h = m*128+p, col r = t*NC + n (host transposes)
    out_d = nc.dram_tensor("h_out", [H, R], FP32, kind="ExternalOutput").ap()
    out_r = out_d.rearrange("(m p) r -> p m r", p=128)

    with tile.TileContext(nc) as tc:
        with (
            tc.tile_pool(name="const", bufs=1) as cp,
            tc.tile_pool(name="work", bufs=2) as wp,
            tc.tile_pool(name="ps", bufs=2, space="PSUM") as pp,
        ):
            ident = cp.tile([128, 128], FP16, tag="ident")
            make_identity(nc, ident)

            # Weights + x cast fp32->fp16 in-flight by SWDGE into write-once
            # buffers.
            wh = []
            for k in range(4):
                whk = cp.tile([128, H], FP16, tag=f"wh{k}", name=f"wh{k}")
                nc.gpsimd.dma_start(whk, wh_d[k * 128 : (k + 1) * 128, :])
                wh.append(whk)
            wi = []
            for k in range(2):
                wik = cp.tile([128, H], FP16, tag=f"wi{k}", name=f"wi{k}")
                nc.gpsimd.dma_start(wik, wi_d[k * 128 : (k + 1) * 128, :])
                wi.append(wik)
            bh = cp.tile([128, 4], FP32, tag="bh")
            nc.gpsimd.dma_start(bh, bh_d.rearrange("(m p) -> p m", p=128))

            # all of x, fp16, row-tile-major: 64 tiles of [128, D]
            xall = cp.tile([128, (R // 128) * D], FP16, tag="xall")
            xall_r = xall.rearrange("p (rt d) -> p rt d", d=D)
            x_r = x_d.rearrange("(rt p) d -> p rt d", p=128)
            for f in range(NF):
                nc.gpsimd.dma_start(
                    xall_r[:, f * 4 : (f + 1) * 4, :], x_r[:, f * 4 : (f + 1) * 4, :]
                )

            # xi^T, m-major on the free axis: [:, m*R + t*NC + n]
            xi = cp.tile([128, 4 * R], FP16, tag="xi")
            xi_r = xi.rearrange("p (m r) -> p m r", m=4)
            dscr = cp.tile([128, NF], FP16, tag="dscr")
            ascr = cp.tile([128, NST], FP16, tag="ascr")

            # One-time observers: PE observes Pool (ident) and the wi DMA
            # lanes; DVE observes the bh DMA lane.
            nc.tensor.ldweights(ident)
            nc.tensor.ldweights(wi[0][:, :128])
            nc.tensor.ldweights(wi[1][:, :128])
            bhobs = cp.tile([128, 4], FP32, tag="bhobs")
            nc.vector.tensor_copy(bhobs, bh)

            # ---- xi chunk computation, one work item at a time ----
            # Chunk f work items (each emits <=1 PE instruction):
            #   0..7 : transpose x tile (2 kd * 4 rt) + DVE copy to xts
            #   8..15: xi matmul (4 m * 2 kd); after each m's 2nd mm, DVE
            #          bias-add into xi chunk f.
            chunk_state = {}

            def chunk_item(f, it):
                if it == 0:
                    xts = []
                    for kd in range(2):
                        xt = wp.tile(
                            [128, FCH], FP16, tag=f"xT{kd}", name=f"xT{kd}_{f}"
                        )
                        xts.append(xt)
                    chunk_state[f] = xts
                    prev = chunk_state.pop(f - 2, None)
                    if prev is not None:
                        # DVE observes its own sem at the last copy of f-2 so
                        # the ring-WAW self-waits of this chunk's copies elide.
                        nc.vector.tensor_copy(
                            dscr[:, f : f + 1], prev[1][:, FCH - 1 :]
                        )
                xts = chunk_state[f]
                if it < 8:
                    rt, kd = divmod(it, 2)
                    base = (f * 4 + rt) * D
                    tp = pp.tile(
                        [128, 128], FP16, tag="tp", bufs=4, name=f"tp{f}_{rt}_{kd}"
                    )
                    nc.tensor.transpose(
                        tp, xall[:, base + kd * 128 : base + (kd + 1) * 128], ident
                    )
                    nc.vector.tensor_copy(xts[kd][:, rt * 128 : (rt + 1) * 128], tp)
                else:
                    m, kd = divmod(it - 8, 2)
                    if kd == 0:
                        chunk_state[(f, "xps", m)] = pp.tile(
                            [128, FCH], FP32, tag="xips", name=f"xps{f}_{m}"
                        )
                    xps = chunk_state[(f, "xps", m)]
                    nc.tensor.matmul(
                        xps,
                        wi[kd][:, m * 128 : (m + 1) * 128],
                        xts[kd],
                        start=(kd == 0),
                        stop=(kd == 1),
                    )
                    if kd == 1:
                        del chunk_state[(f, "xps", m)]
                        nc.vector.tensor_scalar_add(
                            xi[:, m * R + f * FCH : m * R + (f + 1) * FCH],
                            xps,
                            bh[:, m : m + 1],
                        )

            # prologue: chunks 0 and 1 in full
            for f in range(2):
                for it in range(16):
                    chunk_item(f, it)

            # ---- recurrence + interleaved chunk work + output DMA ----
            z_cur = None
            h_prev = None
            h_acc_prev = None
            for s in range(NST):
                if s >= 2:
                    # ACT observes its own sem at the last tanh of s-1 so the
                    # hacc ring WAW of this stage's first tanh elides. Tanh
                    # (not Identity) keeps the ACT table loaded once.
                    nc.scalar.activation(
                        ascr[:, s : s + 1], h_acc_prev[:, 4 * FCH - 1 :], AF.Tanh
                    )
                h_acc = wp.tile([128, 4 * FCH], FP16, tag="hacc", name=f"hacc{s}")
                h_acc_r = h_acc.rearrange("p (m fc) -> p m fc", m=4)
                for tl in range(TST):
                    t = s * TST + tl
                    if t > 0:
                        for m in range(4):
                            for k in range(4):
                                nc.tensor.matmul(
                                    z_cur[:, m * 16 : (m + 1) * 16],
                                    wh[k][:, m * 128 : (m + 1) * 128],
                                    h_prev(k),
                                    start=False,
                                    stop=(k == 3),
                                    skip_group_check=True,
                                )
                    # prefill z for step t+1 via PE (ident stationary, xi
                    # moving, start=True): the whole prefill+accumulate chain
                    # stays on one engine, and emitting it after step t's MMs
                    # lets their ACT wait cover the z-slot read WAR.
                    if t + 1 < L:
                        z_next = pp.tile([128, 64], FP32, tag="z", name=f"z{t + 1}")
                        nc.tensor.matmul(
                            z_next,
                            ident,
                            xi_r[:, :, (t + 1) * NC : (t + 2) * NC],
                            start=True,
                            stop=False,
                            skip_group_check=True,
                        )
                    else:
                        z_next = None
                    # dribble chunk (s+2)'s work into the tanh-wait window,
                    # one item per two steps
                    if s + 2 < NF and tl % 2 == 0:
                        chunk_item(s + 2, tl // 2)
                    out_sl = h_acc_r[:, :, tl * NC : (tl + 1) * NC]
                    if t == 0:
                        nc.scalar.activation(out_sl, xi_r[:, :, :NC], AF.Tanh)
                    else:
                        nc.scalar.activation(
                            out_sl, z_cur.rearrange("p (m w) -> p m w", m=4), AF.Tanh
                        )
                    h_prev = (
                        lambda ha, tl_: lambda k: ha[
                            :, k * FCH + tl_ * NC : k * FCH + (tl_ + 1) * NC
                        ]
                    )(h_acc, tl)
                    z_cur = z_next

                # ---- output DMA for stage s (h^T layout, cast to fp32) ----
                nc.gpsimd.dma_start(
                    out_r[:, :, s * FCH : (s + 1) * FCH],
                    h_acc_r,
                )
                h_acc_prev = h_acc
    _split_waits(nc)
    return nc


def _split_waits(nc):
    # Walrus accepts at most one sem wait per instruction, but the TileContext
    # end-of-program drain aggregates every sem's terminal value. Split any
    # multi-wait instruction into a chain of single-wait drains ahead of it
    # (same engine, in-order issue => identical semantics).
    for f in nc.m.functions:
        for blk in f.blocks:
            insts = list(blk.instructions)
            out = []
            changed = False
            for ins in insts:
                si = ins.sync_info
                w = list(si.on_wait) if si is not None else []
                if len(w) > 1:
                    changed = True
                    for k, sw in enumerate(w[:-1]):
                        nd = mybir.InstDrain(name=f"{ins.name}-w{k}", ins=[], outs=[])
                        nd.engine = ins.engine
                        nd.sync_info = mybir.SyncInfo(on_wait=[sw], on_update=[])
                        out.append(nd)
                    ins.sync_info = mybir.SyncInfo(
                        on_wait=[w[-1]], on_update=list(ins.sync_info.on_update)
                    )
                out.append(ins)
            if changed:
                blk.instructions = out


def _get_nc():
    if "nc" not in _cache:
        _cache["nc"] = _build()
    return _cache["nc"]


def run(inputs, **spmd_kwargs):
    x = np.ascontiguousarray(np.asarray(inputs["x"], dtype=np.float32))
    w_i = np.ascontiguousarray(np.asarray(inputs["w_i"], dtype=np.float32))
    w_h = np.ascontiguousarray(np.asarray(inputs["w_h"], dtype=np.float32))
    b_h = np.ascontiguousarray(np.asarray(inputs["b_h"], dtype=np.float32))
    in_maps = []
    for c in range(NCORES):
        xs = np.ascontiguousarray(x[:, c * NC : (c + 1) * NC, :]).reshape(R, D)
        in_maps.append({"x": xs, "w_i": w_i, "w_h": w_h, "b_h": b_h})
    res = run_bass_kernel_spmd(_get_nc(), in_maps, list(range(NCORES)), **spmd_kwargs)
    out = np.empty((L, N, H), np.float32)
    for c in range(NCORES):
        # device output is h^T [H, R]; transpose back on the host
        out[:, c * NC : (c + 1) * NC, :] = (
            res.results[c]["h_out"].T.reshape(L, NC, H)
        )
    return out, res


def kernel(**inputs) -> np.ndarray:
    out, _ = run(inputs)
    return out
